# revision 26
# baseline (speedup 1.0000x reference)
# Multi-head attention (B=4, S=2048, D=512, H=8) on 8 Trainium2 NeuronCores.
#
# Sharding: core c handles batch c//2 and query rows [(c%2)*1024, (c%2+1)*1024)
# for all 8 heads over all 2048 keys. Output slices are disjoint -> no
# collectives needed.
#
# Final (v7): 165.5us HW exec, rel err 1.02e-2 (baseline was 185.6us).
# Key techniques:
#   - exp split across ScalarE (ACT exp, mask bias on the bias port) and
#     VectorE (Schraudolph bf16 bit-trick via tensor_scalar -> int16 bitcast;
#     softmax ratio cancels the trick's mean bias; ACT side gets +ln(1.0308)
#     so both populations share the same scale). Host-validated ~6e-3.
#   - reciprocal_approx_fast on PSUM rows 0..64 (1-partition APs misbehave),
#     PSUM evac + normalize fused into one scalar_tensor_tensor per head.
#   - Q/K projections in fp8e4 DoubleRow (contraction 256/step, half the
#     matmuls); weights scaled x32 on host so fp8 stays in normal range,
#     the exp scale absorbs the 1/1024.
#   - bv folded into bo on host (attention rows sum to 1), dropping the
#     V-projection bias matmuls.
#   - big inputs only on the scalar/sync HWDGE rings (gpsimd SWDGE is ~8x
#     slower); PE warm-up matmul burst keeps HAM at full clock through the
#     load phase.
#   - attention inner loop software-pipelined: scores(sk+1) issues before
#     attn@v(sk) (both gate on exp(sk)), scores matmuls interleaved A,B so
#     the two PE row-group halves run concurrently; exp(A)->ScalarE,
#     exp(B)->VectorE per sk so the engines work in parallel.
# Known remaining headroom (for a future session): per-matmul cost is
# ~240-310ns vs 213 ideal (walrus emits LDWEIGHTS per matmul; only hidden
# when the next stationary targets a different PE row group); the 8-bank
# PSUM budget (4 score + 4 accumulator) blocks double-buffering the score
# tiles, so each sk cycle pays exp+scores serially (~2.2us); the output
# projection runs at half PE clock after the pair-3 normalize gap; K/V
# projections are computed redundantly by core pairs sharing a batch.

import sys
import os

for _p in ("/opt/trn_rl_repo", "/root/.axon_site/_ro/trn_rl_repo"):
    if os.path.isdir(_p) and _p not in sys.path:
        sys.path.append(_p)

import numpy as np

B, S, D, H = 4, 2048, 512, 8
DK = D // H          # 64
N_CORES = 8
SQ = S // 2          # 1024 query rows per core
SKC = 1152           # compacted key capacity (9 tiles of 128)
NCLEAN = 7           # leading key tiles guaranteed mask-free in compact mode
MASK_BIAS = -50.0
ACT_CENTER = 0.030787    # ln(mean bit-trick scale): centers ACT vs DVE exp
WSCALE = 32.0            # fp8 q/k weight prescale (host)
SSCALE = 0.125 / (WSCALE * WSCALE)            # score scale incl. 1/sqrt(dk)
EXP_C1 = SSCALE * 1.4426950408889634 * 128.0  # scale * log2(e) * 2^7
EXP_C2 = 16256.0                              # 127 * 128 (bf16 exponent bias)
N_WARM = 12          # PE warm-up matmuls riding out the input DMA window

_compiled = {}       # skeys -> Bacc
last_results = None  # BassKernelResults of the most recent run (for test.py)


def _build(skeys, n_clean):
    import concourse.bass as bass  # noqa: F401
    from concourse import bacc
    import concourse.tile as tile
    import concourse.mybir as mybir
    from concourse.alu_op_type import AluOpType

    fp32 = mybir.dt.float32
    bf16 = mybir.dt.bfloat16
    f8 = mybir.dt.float8e4
    i16 = mybir.dt.int16
    DR = mybir.MatmulPerfMode.DoubleRow
    nkt = skeys // 128
    # (sk, half) pairs the DVE handles; all others go to ScalarE ACT. Masked
    # tiles (sk >= n_clean) must use ACT (mask bias rides the ACT bias port).
    dve_exp = {(sk, 1) for sk in range(1, n_clean)}
    kdbg = set(os.environ.get("KDBG", "").split(","))
    if "noexp" in kdbg:
        dve_exp = set()

    nc = bacc.Bacc("TRN2", target_bir_lowering=False, debug=False,
                   num_devices=N_CORES)

    # fp8 DoubleRow layouts: contraction index c = 256*s + 128*ko + ki
    xq = nc.dram_tensor("xq", [128, 2, 2, SQ], f8, kind="ExternalInput")
    xk = nc.dram_tensor("xk", [128, 2, 2, skeys], f8, kind="ExternalInput")
    xv = nc.dram_tensor("xv", [D, skeys], bf16, kind="ExternalInput")
    wq = nc.dram_tensor("wq", [128, 2, 2, D], f8, kind="ExternalInput")
    wk = nc.dram_tensor("wk", [128, 2, 2, D], f8, kind="ExternalInput")
    wv = nc.dram_tensor("wv", [D, D], bf16, kind="ExternalInput")
    wo = nc.dram_tensor("wo", [D, D], bf16, kind="ExternalInput")
    bq = nc.dram_tensor("bq", [128, 4], fp32, kind="ExternalInput")
    bk = nc.dram_tensor("bk", [128, 4], fp32, kind="ExternalInput")
    bo = nc.dram_tensor("bo", [1, D], bf16, kind="ExternalInput")
    mb = nc.dram_tensor("mb", [128, nkt], fp32, kind="ExternalInput")
    out = nc.dram_tensor("out", [SQ, D], bf16, kind="ExternalOutput")
    rds = nc.dram_tensor("rds", [H, SQ], fp32)  # scratch: 1/denominator

    with tile.TileContext(nc) as tc:
        with (
            tc.tile_pool(name="consts", bufs=1) as consts,
            tc.tile_pool(name="xfull", bufs=1) as xfull,
            tc.tile_pool(name="qk", bufs=1) as qk,
            tc.tile_pool(name="vp", bufs=1) as vp,
            tc.tile_pool(name="stp", bufs=6) as stp,
            tc.tile_pool(name="nrm", bufs=6) as nrm,
            tc.tile_pool(name="osb", bufs=2) as osb,
            tc.tile_pool(name="pst", bufs=2, space="PSUM") as pst,
            tc.tile_pool(name="pout", bufs=2, space="PSUM") as pout,
        ):
            # ---- constant / input loads, spread over the DMA rings ----
            wq_sb = consts.tile([128, 2, 2, D], f8, tag="wq")
            wk_sb = consts.tile([128, 2, 2, D], f8, tag="wk")
            wv_sb = consts.tile([128, 4, D], bf16, tag="wv")
            wo_sb = consts.tile([128, 4, D], bf16, tag="wo")
            bq_sb = consts.tile([128, 4], fp32, tag="bq")
            bk_sb = consts.tile([128, 4], fp32, tag="bk")
            bo_sb = consts.tile([1, D], bf16, tag="bo")
            mb_sb = consts.tile([128, nkt], fp32, tag="mb")
            ones_sb = consts.tile([1, 128], bf16, tag="ones")
            nc.vector.memset(ones_sb[:], 1.0)

            xq_sb = xfull.tile([128, 2, 2, SQ], f8, tag="xq")
            xk_sb = xfull.tile([128, 2, 2, skeys], f8, tag="xk")
            xv_sb = xfull.tile([128, 4, skeys], bf16, tag="xv")

            # loads: scalar ring feeds the V path first, sync ring the rest;
            # gpsimd SWDGE only carries the small constants
            nc.scalar.dma_start(out=wv_sb[:],
                                in_=wv.rearrange("(kc p) n -> p kc n", p=128))
            nc.gpsimd.dma_start(out=bq_sb[:], in_=bq[:, :])
            nc.gpsimd.dma_start(out=bk_sb[:], in_=bk[:, :])
            nc.gpsimd.dma_start(out=bo_sb[:], in_=bo[:, :])
            nc.gpsimd.dma_start(out=mb_sb[:], in_=mb[:, :])
            nc.scalar.dma_start(
                out=xv_sb[:, 0:2, :],
                in_=xv[0:256, :].rearrange("(kc p) s -> p kc s", p=128))
            nc.sync.dma_start(
                out=xv_sb[:, 2:4, :],
                in_=xv[256:512, :].rearrange("(kc p) s -> p kc s", p=128))
            nc.sync.dma_start(out=wk_sb[:], in_=wk[:, :, :, :])
            nc.scalar.dma_start(out=xk_sb[:], in_=xk[:, :, :, :])
            nc.sync.dma_start(out=wq_sb[:], in_=wq[:, :, :, :])
            nc.sync.dma_start(out=xq_sb[:], in_=xq[:, :, :, :])
            nc.scalar.dma_start(out=wo_sb[:],
                                in_=wo.rearrange("(j p) n -> p j n", p=128))

            # ACT exp table warm-up (overlaps the DMA window)
            warm = consts.tile([1, 16], fp32, tag="warm")
            nc.vector.memset(warm[:], 0.0)
            nc.scalar.activation(out=warm[:], in_=warm[:],
                                 func=mybir.ActivationFunctionType.Exp)

            # PE warm-up: keep the HAM busy window hot while inputs land
            wmv = consts.tile([1, 512], bf16, tag="wmv")
            nc.vector.memset(wmv[:], 0.0)
            pwarm = pst.tile([128, SQ], fp32, tag="st")
            for _i in range(N_WARM):
                nc.tensor.matmul(pwarm[:, 0:512], ones_sb[:, 0:128],
                                 wmv[:], start=True, stop=True)

            # key-side free-dim chunks of up to 512 (PSUM bank limit)
            kchunks = []
            off = 0
            while off < skeys:
                w = min(512, skeys - off)
                kchunks.append((off, w))
                off += w

            # ---- v projection: v = value @ WvT + bv, per head [v_h | 1] ----
            v_sb = vp.tile([128, nkt, H, DK + 1], bf16, tag="v")
            nc.vector.memset(v_sb[:, :, :, DK:DK + 1], 1.0)
            # (bv is folded into bo on the host: attention rows sum to 1)
            for sk in range(nkt):
                p = pst.tile([128, SQ], fp32, tag="st")
                for kc in range(4):
                    nc.tensor.matmul(
                        p[:, 0:512],
                        xv_sb[:, kc, sk * 128:(sk + 1) * 128],
                        wv_sb[:, kc, :],
                        start=(kc == 0), stop=(kc == 3))
                nc.scalar.copy(
                    out=v_sb[:, sk, :, 0:DK],
                    in_=p[:, 0:512].rearrange("p (h m) -> p h m", h=H))

            qT_sb = qk.tile([128, 4, SQ], bf16, tag="qT")
            kT_sb = qk.tile([128, 4, skeys], bf16, tag="kT")
            outTn_sb = qk.tile([128, 4, SQ], bf16, tag="outTn")

            def proj_j(j):
                # kT[j], qT[j] in fp8 DoubleRow: two 256-deep steps
                for off, w in kchunks:
                    p = pst.tile([128, SQ], fp32, tag="st")
                    for st_ in range(2):
                        nc.tensor.matmul(
                            p[:, 0:w],
                            wk_sb[:, st_, :, j * 128:(j + 1) * 128],
                            xk_sb[:, st_, :, off:off + w],
                            start=(st_ == 0), stop=(st_ == 1), perf_mode=DR)
                    nc.scalar.add(kT_sb[:, j, off:off + w],
                                  p[:, 0:w], bk_sb[:, j:j + 1])
                for qc in range(SQ // 512):
                    p = pst.tile([128, SQ], fp32, tag="st")
                    for st_ in range(2):
                        nc.tensor.matmul(
                            p[:, 0:512],
                            wq_sb[:, st_, :, j * 128:(j + 1) * 128],
                            xq_sb[:, st_, :, qc * 512:(qc + 1) * 512],
                            start=(st_ == 0), stop=(st_ == 1), perf_mode=DR)
                    nc.scalar.add(qT_sb[:, j, qc * 512:(qc + 1) * 512],
                                  p[:, 0:512], bq_sb[:, j:j + 1])

            proj_j(0)
            if "noil" in kdbg:
                for _j in range(1, 4):
                    proj_j(_j)

            # ---- attention, one head pair at a time ----
            # pending_nrm: normalize work of the previous pair, drained one
            # op per early sk of this pair so the DMA broadcast round-trip
            # never stalls the DVE queue ahead of this pair's exps
            pending_nrm = []
            for j in range(4):
                po0 = pout.tile([128, SQ], fp32, tag="po")
                po1 = pout.tile([128, SQ], fp32, tag="po")

                def scores(sk):
                    psA = pst.tile([128, SQ], fp32, tag="st")
                    psB = pst.tile([128, SQ], fp32, tag="st")
                    for qc in range(SQ // 512):
                        nc.tensor.matmul(
                            psA[:, qc * 512:(qc + 1) * 512],
                            kT_sb[0:DK, j, sk * 128:(sk + 1) * 128],
                            qT_sb[0:DK, j, qc * 512:(qc + 1) * 512],
                            start=True, stop=True, tile_position=(0, 0))
                        nc.tensor.matmul(
                            psB[:, qc * 512:(qc + 1) * 512],
                            kT_sb[DK:128, j, sk * 128:(sk + 1) * 128],
                            qT_sb[DK:128, j, qc * 512:(qc + 1) * 512],
                            start=True, stop=True, tile_position=(64, 0))
                    return psA, psB

                def exps(sk, psA, psB):
                    sts = []
                    for half, ps in ((0, psA), (1, psB)):
                        st = stp.tile([128, SQ], bf16, tag="stb")
                        if (sk, half) in dve_exp:
                            nc.vector.tensor_scalar(
                                out=st[:].bitcast(i16), in0=ps[:],
                                scalar1=EXP_C1, scalar2=EXP_C2,
                                op0=AluOpType.mult, op1=AluOpType.add)
                        else:
                            nc.scalar.activation(
                                out=st[:], in_=ps[:],
                                func=mybir.ActivationFunctionType.Exp,
                                bias=mb_sb[:, sk:sk + 1], scale=SSCALE)
                        sts.append(st)
                    return sts

                def attnv(sk, sts):
                    for qc in range(SQ // 512):
                        nc.tensor.matmul(
                            po0[0:DK + 1, qc * 512:(qc + 1) * 512],
                            v_sb[:, sk, 2 * j, :],
                            sts[0][:, qc * 512:(qc + 1) * 512],
                            start=(sk == 0), stop=(sk == nkt - 1))
                    for qc in range(SQ // 512):
                        nc.tensor.matmul(
                            po1[0:DK + 1, qc * 512:(qc + 1) * 512],
                            v_sb[:, sk, 2 * j + 1, :],
                            sts[1][:, qc * 512:(qc + 1) * 512],
                            start=(sk == 0), stop=(sk == nkt - 1))

                # pipelined: scores(sk+1) issues before attn@v(sk) so the
                # next exp starts while attn@v runs (both gate on exp(sk))
                prev = None
                drain = pending_nrm
                pending_nrm = []
                for sk in range(nkt):
                    ps_pair = scores(sk)
                    if prev is not None:
                        attnv(prev[0], prev[1])
                    sts = exps(sk, *ps_pair)
                    if sk < len(drain):
                        drain[sk]()
                    prev = (sk, sts)
                attnv(prev[0], prev[1])

                # normalization: fast-reciprocal the PSUM denom row
                # (over rows 0..64 - 1-partition APs misbehave; only row
                # 64 is consumed), DRAM round-trip partition-broadcast,
                # fused evac+multiply per head straight out of PSUM
                def norm_recip(h, po):
                    rden = nrm.tile([128, SQ], fp32, tag="rden")
                    nc.vector.reciprocal_approx_fast(
                        out=rden[0:DK + 1, :], in_=po[0:DK + 1, :])
                    nc.sync.dma_start(out=rds[h:h + 1, :],
                                      in_=rden[DK:DK + 1, :])
                    bcn = nrm.tile([DK, SQ], fp32, tag="bcn")
                    nc.gpsimd.dma_start(
                        out=bcn[:],
                        in_=rds[h:h + 1, :].partition_broadcast(DK))
                    return bcn

                def norm_mul(j_, half, po, bcn):
                    if half == 0:
                        nc.vector.scalar_tensor_tensor(
                            out=outTn_sb[0:DK, j_, :], in0=po[0:DK, :],
                            scalar=1.0, in1=bcn[:],
                            op0=AluOpType.mult, op1=AluOpType.mult)
                    else:
                        todd = nrm.tile([DK, SQ], bf16, tag="todd")
                        nc.vector.scalar_tensor_tensor(
                            out=todd[:], in0=po[0:DK, :],
                            scalar=1.0, in1=bcn[:],
                            op0=AluOpType.mult, op1=AluOpType.mult)
                        nc.sync.dma_start(out=outTn_sb[DK:128, j_, :],
                                          in_=todd[:])

                bcn0 = norm_recip(2 * j, po0)
                if j < 3:
                    hold = {}
                    pending_nrm = [
                        lambda j_=j, p_=po1, h_=hold:
                            h_.__setitem__('b', norm_recip(2 * j_ + 1, p_)),
                        lambda j_=j, p_=po0, b_=bcn0: norm_mul(j_, 0, p_, b_),
                        lambda j_=j, p_=po1, h_=hold:
                            norm_mul(j_, 1, p_, h_['b']),
                    ]
                else:
                    bcn1 = norm_recip(2 * j + 1, po1)
                    norm_mul(j, 0, po0, bcn0)
                    norm_mul(j, 1, po1, bcn1)

                if j < 3 and "noil" not in kdbg:
                    proj_j(j + 1)

            # ---- output projection ----
            # bo broadcast to all partitions once; bias added on the DVE
            # during evac instead of 8 extra matmuls on the PE
            bo_bc = consts.tile([128, D], bf16, tag="bo_bc")
            nc.gpsimd.dma_start(out=bo_bc[:],
                                in_=bo[0:1, :].partition_broadcast(128))
            # head-outer over 4-tile waves: the first contractions can run
            # while the last head's normalization is still in flight
            # pst-based accumulators first: they only gate on the last exp,
            # so their jj=0..2 contractions fill the pair-3 normalize wait
            # (keeping the PE clock-gate warm); pf-outer order lets each
            # accumulator run as soon as its own dependencies clear
            for wave in range(2):
                pfs = []
                for i in range(2):
                    pfs.append(pst.tile([128, SQ], fp32, tag="st",
                                        name=f"pf_{wave}_{i}"))
                for i in range(2):
                    pfs.append(pout.tile([128, SQ], fp32, tag="po",
                                         name=f"pf_{wave}_{i + 2}"))
                for i, pf in enumerate(pfs):
                    sq = wave * 4 + i
                    for jj in range(4):
                        nc.tensor.matmul(
                            pf[:, 0:512],
                            outTn_sb[:, jj, sq * 128:(sq + 1) * 128],
                            wo_sb[:, jj, :],
                            start=(jj == 0), stop=(jj == 3))
                for i, pf in enumerate(pfs):
                    ob = osb.tile([128, 512], bf16, tag="ob")
                    nc.vector.tensor_add(out=ob[:], in0=pf[:, 0:512],
                                         in1=bo_bc[:])
                    sq = wave * 4 + i
                    nc.sync.dma_start(out=out[sq * 128:(sq + 1) * 128, :],
                                      in_=ob[:])

    nc.finalize()
    return nc


def _get_nc(skeys, n_clean):
    key = (skeys, n_clean)
    if key not in _compiled:
        _compiled[key] = _build(skeys, n_clean)
    return _compiled[key]


def _dr_pack(a):
    # [512, N] -> [ki=128, s=2, ko=2, N] with c = 256*s + 128*ko + ki
    n = a.shape[1]
    return np.ascontiguousarray(a.reshape(2, 2, 128, n).transpose(2, 0, 1, 3))


def kernel(query, key, value, key_padding_mask, Wq, bq, Wk, bk, Wv, bv,
           Wo, bo):
    global last_results
    from concourse.bass_utils import run_bass_kernel_spmd
    import ml_dtypes
    bf = ml_dtypes.bfloat16
    e4 = ml_dtypes.float8_e4m3fn

    query = np.asarray(query, dtype=np.float32)
    key = np.asarray(key, dtype=np.float32)
    value = np.asarray(value, dtype=np.float32)
    mask = np.asarray(key_padding_mask).astype(bool)
    Wq = np.asarray(Wq, dtype=np.float32)
    Wk = np.asarray(Wk, dtype=np.float32)
    Wv = np.asarray(Wv, dtype=np.float32)
    Wo = np.asarray(Wo, dtype=np.float32)
    bqv = np.asarray(bq, dtype=np.float32) * np.float32(WSCALE)
    bkv = np.asarray(bk, dtype=np.float32) * np.float32(WSCALE)
    bvv = np.asarray(bv, dtype=np.float32)
    bov = np.asarray(bo, dtype=np.float32) + bvv @ Wo.T  # bv folded in

    # compact keys: keep only unmasked positions (padded to SKC); dense
    # fallback when a batch keeps more than SKC or fewer than NCLEAN*128
    kept = [np.flatnonzero(~mask[b]) for b in range(B)]
    if (max(len(k) for k in kept) <= SKC
            and min(len(k) for k in kept) >= NCLEAN * 128):
        skeys, n_clean = SKC, NCLEAN
        kidx = []
        mbias = []
        for b in range(B):
            idx = np.zeros(SKC, dtype=np.int64)
            idx[:len(kept[b])] = kept[b]
            kidx.append(idx)
            mbias.append(np.where(np.arange(SKC) < len(kept[b]),
                                  np.float32(0.0), np.float32(MASK_BIAS)))
    else:
        skeys, n_clean = S, 0
        kidx = [None] * B
        mbias = [np.where(mask[b], np.float32(MASK_BIAS), np.float32(0.0))
                 for b in range(B)]

    nc = _get_nc(skeys, n_clean)
    nkt = skeys // 128

    shared = {
        "wq": _dr_pack((Wq.T * WSCALE).astype(np.float32)).astype(e4),
        "wk": _dr_pack((Wk.T * WSCALE).astype(np.float32)).astype(e4),
        "wv": np.ascontiguousarray(Wv.T).astype(bf),
        "wo": np.ascontiguousarray(Wo.T).astype(bf),
        "bq": np.ascontiguousarray(bqv.reshape(4, 128).T),
        "bk": np.ascontiguousarray(bkv.reshape(4, 128).T),
        "bo": bov.reshape(1, D).astype(bf),
    }
    in_maps = []
    for c in range(N_CORES):
        b, qh = divmod(c, 2)
        kc_ = key[b] if kidx[b] is None else key[b][kidx[b]]
        vc_ = value[b] if kidx[b] is None else value[b][kidx[b]]
        qT = np.ascontiguousarray(query[b].T)
        m = {
            "xq": _dr_pack(np.ascontiguousarray(
                qT[:, qh * SQ:(qh + 1) * SQ])).astype(e4),
            "xk": _dr_pack(np.ascontiguousarray(kc_.T)).astype(e4),
            "xv": np.ascontiguousarray(vc_.T).astype(bf),
            "mb": np.ascontiguousarray(
                (mbias[b] + np.float32(ACT_CENTER)).reshape(nkt, 128).T),
        }
        m.update(shared)
        in_maps.append(m)

    res = run_bass_kernel_spmd(nc, in_maps, list(range(N_CORES)))
    last_results = res

    out = np.empty((B, S, D), dtype=np.float32)
    for c in range(N_CORES):
        b, qh = divmod(c, 2)
        out[b, qh * SQ:(qh + 1) * SQ, :] = \
            res.results[c]["out"].astype(np.float32)
    return out
